# revision 11
# baseline (speedup 1.0000x reference)
"""GNN message-passing (scatter-add) kernel for 8 Trainium2 NeuronCores.

Computes out = segment_sum(x[src], dst, num_segments=N) for
x [10000, 128] f32, edge_index [2, 320000] int64.

Strategy — dense count-matrix matmul (no gathers, no collectives):
  out[d] = sum_s A[s, d] * x[s]   with A[s, d] = #edges s->d.

  - Host computes A (np.bincount over (src, dst) pairs) and shards its
    columns: core c owns dst range [c*1264, (c+1)*1264). A entries are
    small ints, exact in fp8e4 (<=16); larger counts split into extra
    passes (never triggers for random graphs).
  - On device, out^T[f, d] = sum_k x_k^T-stationary @ A_k-moving: the
    contraction runs over 79 source-node chunks of 128 on the PE, with
    x chunk [128 s, 128 f] fp16 stationary and A chunks [128 s, w d]
    fp8 moving (d-tiles of 512|512|240) accumulating f32 into three
    persistent PSUM banks (mixed fp16 x fp8 matmul is exact here).
  - x is PACKED INTO the A stream: each chunk's DRAM row is 1520 bytes
    per partition = 1264 A bytes + 256 bytes of that source row's fp16
    features. One HWDGE queue carries one stream in exact PE
    consumption order; the stationary is read from the same SBUF tile
    through an fp16 bitcast AP. Group sizes [1, 2, 4, 4, ...] get the
    first matmul issued early; bufs=10 prefetch rides out hiccups.
  - PE warmup: dependency-free matmuls on an uninitialized scratch
    tile run during the first load's DMA window so the HAM clock-gate
    is open (2.4 GHz) when real data arrives.
  - The last group runs tile-major so each PSUM d-tile finishes and
    drains (DVE copy + out DMA on the second HWDGE queue) under the
    remaining matmuls of the other tiles.
  - fp16 x keeps relative L2 error ~2e-4 (resid_var ~4e-8 vs 1e-4
    gate); A is exact, PSUM accumulates in f32.
  - Host transposes/concats the 8 cores' out^T tiles back to [10000, 128].

Per-core traffic: aug stream 15.4MB + out 0.65MB.
"""

import sys

for _p in ("/opt/trn_rl_repo",):
    if _p not in sys.path:
        sys.path.append(_p)

import ml_dtypes
import numpy as np

import concourse.bacc as bacc
import concourse.mybir as mybir
import concourse.tile as tile
import concourse.bass_utils as _bass_utils
from concourse.bass_utils import run_bass_kernel_spmd

# The NEFF epilogue zeroes the whole 256-entry semaphore file one
# EVENT_SEMAPHORE at a time (~7.8us). Capping walrus's semaphore budget
# shrinks that sweep; bass-managed sems live in [150, 256) and are
# untouched by walrus's allocator either way.
if not getattr(_bass_utils.get_walrus_args, "_sem_patched", False):
    _orig_get_walrus_args = _bass_utils.get_walrus_args

    def _patched_get_walrus_args(*a, **k):
        return [*_orig_get_walrus_args(*a, **k), "--max-sem-num=16"]

    _patched_get_walrus_args._sem_patched = True
    _bass_utils.get_walrus_args = _patched_get_walrus_args

N_NODES = 10000
D_FEAT = 128
N_CORES = 8
P = 128
KCH = -(-N_NODES // P)  # 79 source chunks
NPAD = KCH * P  # 10112 (source rows padded; dst needs no padding)
DCORE = NPAD // N_CORES  # 1264 dst columns per core (16B-aligned A rows)
XB = D_FEAT * 2  # 256 bytes of fp16 features per chunk-partition
CHB = DCORE + XB  # 1520 bytes per chunk per partition in the aug stream
DTILES = [(0, 512), (512, 512), (1024, DCORE - 1024)]
KGROUPS = [1, 2, 3, 4] + [4] * 16 + [5]  # source-chunk load groups (sum = 79)
DRAIN_CHUNKS = 7  # tile-major drain spans the last two groups
assert sum(KGROUPS) == KCH
FP8 = ml_dtypes.float8_e4m3
FP8_MAX_INT = 16
N_WARMUP = 6  # dummy PE warmup matmuls (512 cols each)
FILLERS = {0: 6, 1: 2, 2: 2, 3: 1}  # post-group warm fillers (early pipeline ramp)

# test/profiling hooks
TRACE = False
TRACE_CORES = None
LAST_RESULT = None


def _build_program(n_passes: int):
    nc = bacc.Bacc(
        "TRN2", target_bir_lowering=False, debug=False, num_devices=N_CORES
    )
    a_ds = [
        nc.dram_tensor(
            f"a{ip}", [P, KCH * CHB], mybir.dt.float8e4, kind="ExternalInput"
        )
        for ip in range(n_passes)
    ]
    o_d = nc.dram_tensor("o", [P, DCORE], mybir.dt.float32, kind="ExternalOutput")

    gk = []  # (group index, start k, size) triples
    k0 = 0
    for gi, gn in enumerate(KGROUPS):
        gk.append((gi, k0, gn))
        k0 += gn

    with tile.TileContext(nc) as tc:
        with (
            tc.tile_pool(name="warm", bufs=1) as wp,
            tc.tile_pool(name="g1", bufs=1) as p1,
            tc.tile_pool(name="g2", bufs=1) as p2,
            tc.tile_pool(name="g3", bufs=1) as p3,
            tc.tile_pool(name="g4", bufs=12) as p4,
            tc.tile_pool(name="g5", bufs=1) as p5,
            tc.tile_pool(name="res", bufs=4) as resp,
            tc.tile_pool(name="ps", bufs=1, space="PSUM") as psp,
        ):
            pools = {1: p1, 2: p2, 3: p3, 4: p4, 5: p5}
            # PE warmup: dummy matmuls reading an uninitialized scratch
            # tile into a scratch PSUM bank, so the HAM clock-gate opens
            # during the first loads' DMA window.
            warm = wp.tile([P, 512], mybir.dt.float16, tag="warm", name="warm")
            wps = psp.tile([P, 512], mybir.dt.float32, tag="wps", name="wps")
            nc.gpsimd.memset(warm[:], 0.0)
            # prewarm the second HWDGE ring so the drain DMAs at the end
            # don't pay first-use latency
            pre = wp.tile([P, 64], mybir.dt.float8e4, tag="pre", name="pre")
            nc.scalar.dma_start(out=pre[:], in_=a_ds[0][:, 0:64])
            for _ in range(N_WARMUP):
                nc.tensor.matmul(
                    wps[:], warm[:, 0:P], warm[:], start=True, stop=True
                )
            pss = [
                psp.tile([P, w], mybir.dt.float32, tag=f"ps{t}", name=f"ps{t}")
                for t, (off, w) in enumerate(DTILES)
            ]
            mi = 0
            n_mm = n_passes * KCH
            drain_mms = []  # (tile slice of a_sb, kk) pairs for the tail
            for ip in range(n_passes):
                av = a_ds[ip][:].rearrange("p (k c) -> p k c", k=KCH, c=CHB)
                for gi, k0, gn in gk:
                    a_sb = pools[gn].tile(
                        [P, gn, CHB],
                        mybir.dt.float8e4,
                        tag=f"g{gn}",
                        name=f"a{ip}_{gi}",
                    )
                    nc.sync.dma_start(out=a_sb[:], in_=av[:, k0 : k0 + gn, :])
                    for kk in range(gn):
                        if ip == n_passes - 1 and mi >= n_mm - DRAIN_CHUNKS:
                            drain_mms.append((a_sb, kk))
                            mi += 1
                            continue
                        xt = a_sb[:, kk, DCORE:CHB].bitcast(mybir.dt.float16)
                        for t, (off, w) in enumerate(DTILES):
                            nc.tensor.matmul(
                                pss[t][:],
                                xt,
                                a_sb[:, kk, off : off + w],
                                start=(mi == 0),
                                stop=False,
                            )
                        mi += 1
                    # warm filler matmuls bridge the deterministic early
                    # gap between this group's matmuls and the next
                    # group's DMA-completion semaphore, keeping the HAM
                    # clock-gate open instead of idling the PE
                    for _ in range(FILLERS.get(gi, 0) if ip == 0 else 0):
                        nc.tensor.matmul(
                            wps[:], warm[:, 0:P], warm[:], start=True, stop=True
                        )
            # tile-major tail over the final DRAIN_CHUNKS chunks: each PSUM
            # d-tile finishes and drains (DVE copy + out DMA) under the
            # remaining matmuls of the other tiles
            for t, (off, w) in enumerate(DTILES):
                for j, (a_sb, kk) in enumerate(drain_mms):
                    xt = a_sb[:, kk, DCORE:CHB].bitcast(mybir.dt.float16)
                    nc.tensor.matmul(
                        pss[t][:],
                        xt,
                        a_sb[:, kk, off : off + w],
                        start=False,
                        stop=(j == len(drain_mms) - 1),
                    )
                res = resp.tile(
                    [P, w], mybir.dt.float32, tag=f"res{t}", name=f"res{t}"
                )
                nc.vector.tensor_copy(res[:], pss[t][:])
                nc.scalar.dma_start(out=o_d[:, off : off + w], in_=res[:])

    nc.compile()
    return nc


def _prepare(x: np.ndarray, edge_index: np.ndarray):
    ei = np.asarray(edge_index)  # pull to host before any indexing
    src = ei[0].astype(np.int64)
    dst = ei[1].astype(np.int64)

    xf = np.asarray(x).astype(np.float32)
    xp = np.zeros((NPAD, D_FEAT), np.float16)
    xp[:N_NODES] = xf
    # x bytes per chunk-partition: xb[p, k, :] = bytes of x[k*128 + p, :]
    xb = np.ascontiguousarray(
        xp.reshape(KCH, P, D_FEAT).transpose(1, 0, 2)
    ).view(np.uint8).reshape(P, KCH, XB)

    per_core_As = []
    n_passes = 1
    for c in range(N_CORES):
        sel = (dst >= c * DCORE) & (dst < (c + 1) * DCORE)
        idx = src[sel] * DCORE + (dst[sel] - c * DCORE)
        cnt = np.bincount(idx, minlength=NPAD * DCORE).reshape(NPAD, DCORE)
        passes = []
        while True:
            part = np.minimum(cnt, FP8_MAX_INT)
            # [P, KCH, DCORE] chunk-partition layout, viewed as raw bytes
            ab = (
                np.ascontiguousarray(
                    part.astype(FP8).reshape(KCH, P, DCORE).transpose(1, 0, 2)
                )
                .view(np.uint8)
                .reshape(P, KCH, DCORE)
            )
            aug = np.empty((P, KCH, CHB), np.uint8)
            aug[:, :, :DCORE] = ab
            aug[:, :, DCORE:] = xb
            passes.append(
                np.ascontiguousarray(aug.reshape(P, KCH * CHB)).view(FP8)
            )
            cnt = cnt - part
            if not cnt.any():
                break
        per_core_As.append(passes)
        n_passes = max(n_passes, len(passes))

    zeros = None
    in_maps = []
    for c in range(N_CORES):
        m = {}
        for ip in range(n_passes):
            if ip < len(per_core_As[c]):
                m[f"a{ip}"] = per_core_As[c][ip]
            else:
                if zeros is None:
                    zeros = np.zeros((P, KCH * CHB), FP8)
                m[f"a{ip}"] = zeros
        in_maps.append(m)
    return in_maps, n_passes


def kernel(x: np.ndarray, edge_index: np.ndarray) -> np.ndarray:
    global LAST_RESULT
    in_maps, n_passes = _prepare(x, edge_index)
    nc = _build_program(n_passes)
    res = run_bass_kernel_spmd(
        nc,
        in_maps,
        list(range(N_CORES)),
        trace=TRACE,
        trace_cores=TRACE_CORES if TRACE else None,
    )
    LAST_RESULT = res
    # o per core: [128 f, DCORE d] -> out[c*DCORE + d, f]
    out = np.concatenate(
        [np.asarray(r["o"], np.float32).T for r in res.results], axis=0
    )
    return np.ascontiguousarray(out[:N_NODES])


if __name__ == "__main__":
    rng = np.random.default_rng(0)
    x = rng.standard_normal((N_NODES, D_FEAT), dtype=np.float32)
    edge_index = rng.integers(0, N_NODES, size=(2, 320000)).astype(np.int64)
    out = kernel(x, edge_index)
    ref = np.zeros((N_NODES, D_FEAT), np.float32)
    np.add.at(ref, edge_index[1], x[edge_index[0]])
    rel = np.linalg.norm(out - ref) / np.linalg.norm(ref)
    print("rel L2 err:", rel)
